# revision 1
# baseline (speedup 1.0000x reference)
"""GCN (3-layer + global mean pool + FC/sigmoid) on 8 Trainium2 NeuronCores.

Node-sharded graph partitioning: nodes split into 8 contiguous shards of
6250; each core owns its shard's incident edges, grouped by 128-node dst
tile. Aggregation is aggregate-first (A @ h, then @ W): feature-table rows
are fetched with dma_gather (int16 indices over two 25000-row table
halves), scatter-added via one-hot matmuls on the tensor engine, then the
weight matmul runs with the bias folded in as a K=1 matmul. Tables are
bf16 with fp32 PSUM accumulation. h1/h2 and the pooling partials are
exchanged with AllGather collectives; every core then computes the tiny
FC + sigmoid head redundantly.
"""
import sys
import os

for _p in ("/opt/trn_rl_repo", "/root/.axon_site/_ro/trn_rl_repo"):
    if os.path.isdir(_p) and _p not in sys.path:
        sys.path.append(_p)

import numpy as np
import ml_dtypes

bf16 = ml_dtypes.bfloat16

N = 50000
E = 150000
G = 256
NC = 8
SH = N // NC             # 6250 nodes per core
TPC = (SH + 127) // 128  # 49 tiles per core (last tile has 106 nodes)
HALF = N // 2            # table half size for int16 indexing
H1, H2, H3 = 128, 256, 512
SEG = int(os.environ.get('KSEG', '8'))  # chunks per dma_gather call
STAGE = int(os.environ.get('KSTAGE', '6'))

TRACE = False
LAST_EXEC_NS = None
_CACHE = {}


def _prep(x, edge_index, edge_weight, batch):
    """Host-side graph preprocessing -> per-core metadata arrays."""
    x = np.asarray(x, np.float32)
    ei = np.asarray(edge_index)
    ew = np.asarray(edge_weight, np.float32)
    batch = np.asarray(batch).astype(np.int64)

    src = ei[0].astype(np.int64)
    dst = ei[1].astype(np.int64)
    w = ew
    deg = np.bincount(dst, weights=w, minlength=N).astype(np.float32) + 1.0
    dinv = (1.0 / np.sqrt(deg)).astype(np.float32)
    norm = (dinv[src] * w * dinv[dst]).astype(np.float32)
    norm_self = (dinv * dinv).astype(np.float32)  # self-loop weight 1

    core = dst // SH
    tile_l = (dst % SH) // 128
    src_owner = src // SH
    src_loc = src % SH
    half = (src_loc >= SH // 2).astype(np.int64)
    src_tab = src_owner * (SH // 2) + np.where(half == 1, src_loc - SH // 2, src_loc)
    gtile = core * TPC + tile_l
    order = np.lexsort((src, half, gtile))
    src_s, dst_s, norm_s = src_tab[order], dst[order], norm[order]

    key = gtile[order] * 2 + half[order]
    cnt = np.bincount(key, minlength=NC * TPC * 2).reshape(NC, TPC, 2)
    ch_uni = ((cnt + 127) // 128).max(axis=0)   # [TPC, 2]
    CHA = int(ch_uni[:, 0].sum())
    CHB = int(ch_uni[:, 1].sum())

    csumA = np.zeros(TPC + 1, np.int64)
    csumA[1:] = np.cumsum(ch_uni[:, 0])
    csumB = np.zeros(TPC + 1, np.int64)
    csumB[1:] = np.cumsum(ch_uni[:, 1])

    block_start = np.zeros(NC * TPC * 2 + 1, np.int64)
    block_start[1:] = np.cumsum(cnt.reshape(-1))

    cntg = np.bincount(batch, minlength=G).astype(np.float32)
    cntinv_g = (1.0 / np.maximum(cntg, 1.0)).astype(np.float32)

    g0s = [int(batch[c * SH]) for c in range(NC)]
    for c in range(NC):
        assert int(batch[(c + 1) * SH - 1]) - g0s[c] < 128, "graph window > 128"

    def idx_pack(lin):
        a = lin.reshape(-1, 16).T
        return np.ascontiguousarray(np.tile(a, (8, 1)))

    def col_pack(lin):
        return np.ascontiguousarray(lin.reshape(-1, 128).T)

    per_core = []
    for c in range(NC):
        idxA = np.zeros(CHA * 128, np.int16)
        idxB = np.zeros(CHB * 128, np.int16)
        dstlA = np.full(CHA * 128, -1.0, np.float32)
        dstlB = np.full(CHB * 128, -1.0, np.float32)
        normA = np.zeros(CHA * 128, np.float32)
        normB = np.zeros(CHB * 128, np.float32)
        for t in range(TPC):
            for h, (idx_a, dstl_a, norm_a, csum, base) in enumerate(
                ((idxA, dstlA, normA, csumA, 0), (idxB, dstlB, normB, csumB, HALF))
            ):
                bkey = (c * TPC + t) * 2 + h
                b0, b1 = block_start[bkey], block_start[bkey + 1]
                n = b1 - b0
                s0 = int(csum[t]) * 128
                idx_a[s0:s0 + n] = src_s[b0:b1].astype(np.int16)
                dstl_a[s0:s0 + n] = (dst_s[b0:b1] - (c * SH + t * 128)).astype(np.float32)
                norm_a[s0:s0 + n] = norm_s[b0:b1]

        bl = np.full((TPC * 128,), -1.0, np.float32)
        bl[:SH] = (batch[c * SH:(c + 1) * SH] - g0s[c]).astype(np.float32)

        ig = g0s[c] + np.arange(128)
        cinv = np.where(ig < G, cntinv_g[np.minimum(ig, G - 1)], 0.0)

        # precomputed one-hot scatter matrices: oh[p, gc*128 + n] =
        # (dstl[gc*128+p] == n) * norm[gc*128+p]; A-chunks, B-chunks, then
        # one diagonal self-loop chunk per tile
        dstl_cat = np.concatenate([dstlA, dstlB])
        norm_cat = np.concatenate([normA, normB]).astype(bf16).astype(np.float32)
        S = dstl_cat.shape[0]
        oh = np.zeros((S, 128), np.float32)
        valid = dstl_cat >= 0
        oh[np.arange(S)[valid], dstl_cat[valid].astype(np.int64)] = norm_cat[valid]
        oh = oh.reshape(S // 128, 128, 128).transpose(1, 0, 2).reshape(128, S)

        ns_pad = np.zeros(TPC * 128, np.float32)
        ns_pad[:SH] = norm_self[c * SH:(c + 1) * SH]
        selfoh = np.zeros((128, TPC, 128), np.float32)
        pr = np.arange(128)
        for t in range(TPC):
            selfoh[pr, t, pr] = ns_pad[t * 128:(t + 1) * 128]
        selfoh = selfoh.reshape(128, TPC * 128)

        per_core.append(dict(
            idxA=idx_pack(idxA), idxB=idx_pack(idxB),
            ohall=np.ascontiguousarray(
                np.concatenate([oh, selfoh], axis=1)).astype(bf16),
            xTs=np.ascontiguousarray(
                x[c * SH:(c + 1) * SH].T).astype(bf16),
            batchloc=np.ascontiguousarray(bl.reshape(TPC, 128).T),
            cntinv=cinv.astype(np.float32).reshape(128, 1),
        ))

    struct = dict(
        CHA=CHA, CHB=CHB,
        csumA=[int(v) for v in csumA], csumB=[int(v) for v in csumB],
        g0s=g0s,
    )
    return per_core, struct


def _build(struct):
    import concourse.bacc as bacc
    import concourse.mybir as mybir
    import concourse.tile as tile
    from concourse.masks import make_identity

    dt = mybir.dt
    AF = mybir.ActivationFunctionType
    OP = mybir.AluOpType

    CHA, CHB = struct["CHA"], struct["CHB"]
    csumA, csumB = struct["csumA"], struct["csumB"]
    g0s = struct["g0s"]

    nc = bacc.Bacc("TRN2", target_bir_lowering=False, debug=False, num_devices=NC)

    xT_in = nc.dram_tensor("xT", [8, N], dt.bfloat16, kind="ExternalInput")
    w1_in = nc.dram_tensor("w1", [8, H1], dt.bfloat16, kind="ExternalInput")
    w2_in = nc.dram_tensor("w2", [H1, H2], dt.bfloat16, kind="ExternalInput")
    w3_in = nc.dram_tensor("w3", [128, 2, H3], dt.bfloat16, kind="ExternalInput")
    wfc_in = nc.dram_tensor("wfc", [128, 4], dt.float32, kind="ExternalInput")
    b1_in = nc.dram_tensor("b1", [1, H1], dt.bfloat16, kind="ExternalInput")
    b2_in = nc.dram_tensor("b2", [1, H2], dt.bfloat16, kind="ExternalInput")
    b3_in = nc.dram_tensor("b3", [1, H3], dt.bfloat16, kind="ExternalInput")
    bfc_in = nc.dram_tensor("bfc", [1, 1], dt.float32, kind="ExternalInput")
    iota_in = nc.dram_tensor("iota", [128, 128], dt.bfloat16, kind="ExternalInput")
    idxA_in = nc.dram_tensor("idxA", [128, CHA * 8], dt.int16, kind="ExternalInput")
    idxB_in = nc.dram_tensor("idxB", [128, CHB * 8], dt.int16, kind="ExternalInput")
    oh_in = nc.dram_tensor("ohall", [128, (CHA + CHB + TPC) * 128], dt.bfloat16,
                           kind="ExternalInput")
    xTs_in = nc.dram_tensor("xTs", [8, SH], dt.bfloat16, kind="ExternalInput")
    bl_in = nc.dram_tensor("batchloc", [128, TPC], dt.float32, kind="ExternalInput")
    cinv_in = nc.dram_tensor("cntinv", [128, 1], dt.float32, kind="ExternalInput")
    out_ext = nc.dram_tensor("out", [G, 1], dt.float32, kind="ExternalOutput")

    with tile.TileContext(nc) as tc:
        with tc.tile_pool(name="const", bufs=1) as cp, \
             tc.tile_pool(name="meta", bufs=1) as mp, \
             tc.tile_pool(name="gseg", bufs=2) as gp, \
             tc.tile_pool(name="work", bufs=3) as wp, \
             tc.tile_pool(name="slab2", bufs=1) as slp2, \
             tc.tile_pool(name="pps", bufs=1, space="PSUM") as pps, \
             tc.tile_pool(name="dram", bufs=1, space="DRAM") as dram:

            def load(pool, t_in, shape, dtype, tag):
                t = pool.tile(shape, dtype, tag=tag)
                nc.sync.dma_start(t[:], t_in[:])
                return t

            iota_sb = load(cp, iota_in, [128, 128], dt.bfloat16, "iota")
            w1_sb = load(cp, w1_in, [8, H1], dt.bfloat16, "w1")
            w2_sb = load(cp, w2_in, [H1, H2], dt.bfloat16, "w2")
            w3_sb = load(cp, w3_in, [128, 2, H3], dt.bfloat16, "w3")
            wfc_sb = load(cp, wfc_in, [128, 4], dt.float32, "wfc")
            b1_sb = load(cp, b1_in, [1, H1], dt.bfloat16, "b1")
            b2_sb = load(cp, b2_in, [1, H2], dt.bfloat16, "b2")
            b3_sb = load(cp, b3_in, [1, H3], dt.bfloat16, "b3")
            bfc_sb = load(cp, bfc_in, [1, 1], dt.float32, "bfc")
            idx_sbs = [load(mp, idxA_in, [128, CHA * 8], dt.int16, "idxA"),
                       load(mp, idxB_in, [128, CHB * 8], dt.int16, "idxB")]
            oh_all = load(mp, oh_in, [128, (CHA + CHB + TPC) * 128], dt.bfloat16,
                          "ohall")
            xTs_sb = load(mp, xTs_in, [8, SH], dt.bfloat16, "xTs")
            bl_sb = load(mp, bl_in, [128, TPC], dt.float32, "bl")
            cinv_sb = load(mp, cinv_in, [128, 1], dt.float32, "cinv")

            ones_bf = cp.tile([1, 128], dt.bfloat16, tag="ones_bf")
            nc.gpsimd.memset(ones_bf[:], 1.0)
            ones_f32 = cp.tile([1, 128], dt.float32, tag="ones_f32")
            nc.gpsimd.memset(ones_f32[:], 1.0)
            ident = cp.tile([128, 128], dt.float32, tag="ident")
            make_identity(nc, ident[:])

            z1A = dram.tile([HALF, H1], dt.bfloat16, tag="z1A")
            z1B = dram.tile([HALF, H1], dt.bfloat16, tag="z1B")
            h1_shard = dram.tile([SH, H1], dt.bfloat16, tag="h1s")
            h1A = dram.tile([HALF, H1], dt.bfloat16, tag="h1A")
            h1B = dram.tile([HALF, H1], dt.bfloat16, tag="h1B")
            h2_shard = dram.tile([SH, H2], dt.bfloat16, tag="h2s")
            h2A = dram.tile([HALF, H2], dt.bfloat16, tag="h2A")
            h2B = dram.tile([HALF, H2], dt.bfloat16, tag="h2B")
            pool_shard = dram.tile([128, H3], dt.float32, tag="pls")
            pool_all = dram.tile([NC * 128, H3], dt.float32, tag="pla")

            # ---- z1 = x @ W1 for all N nodes (replicated on every core) ----
            with tc.tile_pool(name="psz", bufs=2, space="PSUM") as psz:
                for blk in range(NC):
                    xT_sb = wp.tile([8, SH], dt.bfloat16, tag="xT", bufs=2)
                    nc.sync.dma_start(xT_sb[:], xT_in[:, blk * SH:(blk + 1) * SH])
                    hs = SH // 2  # 3125
                    a0 = blk * hs
                    # half A: tiles 0..24 (rows 0..3124)
                    stgA = wp.tile([128, 25, H1], dt.bfloat16, tag="zstage", bufs=2,
                                   name="stgA")
                    for j in range(25):
                        r = min(128, hs - j * 128)
                        z_ps = psz.tile([128, H1], dt.float32, tag="zps")
                        nc.tensor.matmul(z_ps[:r], lhsT=xT_sb[:, j * 128:j * 128 + r],
                                         rhs=w1_sb[:], start=True, stop=True)
                        if j % 2 == 0:
                            nc.vector.tensor_copy(stgA[:r, j, :], z_ps[:r])
                        else:
                            nc.scalar.activation(stgA[:r, j, :], z_ps[:r], AF.Copy)
                    nc.sync.dma_start(
                        z1A[a0:a0 + 24 * 128, :].rearrange("(j p) f -> p j f", p=128),
                        stgA[:, :24, :])
                    nc.sync.dma_start(z1A[a0 + 24 * 128:a0 + hs, :],
                                      stgA[:hs - 24 * 128, 24, :])
                    # half B: rows 3125..6249 (24 full 128-row tiles + 53 tail)
                    stgB = wp.tile([128, 25, H1], dt.bfloat16, tag="zstage", bufs=2,
                                   name="stgB")
                    for j in range(25):
                        r0 = hs + j * 128
                        r = min(128, SH - r0)
                        if r <= 0:
                            break
                        z_ps = psz.tile([128, H1], dt.float32, tag="zps")
                        nc.tensor.matmul(z_ps[:r], lhsT=xT_sb[:, r0:r0 + r],
                                         rhs=w1_sb[:], start=True, stop=True)
                        if j % 2 == 0:
                            nc.vector.tensor_copy(stgB[:r, j, :], z_ps[:r])
                        else:
                            nc.scalar.activation(stgB[:r, j, :], z_ps[:r], AF.Copy)
                    nc.sync.dma_start(
                        z1B[a0:a0 + 24 * 128, :].rearrange("(j p) f -> p j f", p=128),
                        stgB[:, :24, :])
                    nc.sync.dma_start(z1B[a0 + 24 * 128:a0 + hs, :],
                                      stgB[:hs - 24 * 128, 24, :])

            # ---- self-loop slabs: own-shard rows, fetched without gather ----
            slabs = {}
            with tc.tile_pool(name="psslab", bufs=2, space="PSUM") as pss:
                sl1 = slp2.tile([128, TPC, H1], dt.bfloat16, tag="slab1")
                nc.gpsimd.memset(sl1[:, TPC - 1, :], 0.0)
                for t in range(TPC):
                    r = min(128, SH - t * 128)
                    zs_ps = pss.tile([128, H1], dt.float32, tag="zs")
                    nc.tensor.matmul(zs_ps[:r], lhsT=xTs_sb[:, t * 128:t * 128 + r],
                                     rhs=w1_sb[:], start=True, stop=True)
                    if t % 2 == 0:
                        nc.vector.tensor_copy(sl1[:r, t, :], zs_ps[:r])
                    else:
                        nc.scalar.activation(sl1[:r, t, :], zs_ps[:r], AF.Copy)
                slabs[1] = sl1

            def load_slab(lidx, shard, fdim):
                t = slp2.tile([128, TPC, fdim], dt.bfloat16, tag=f"slab{lidx}",
                              name=f"slab{lidx}")
                nc.gpsimd.memset(t[:, TPC - 1, :], 0.0)
                full = (TPC - 1) * 128
                nc.sync.dma_start(
                    t[:, :TPC - 1, :],
                    shard[:full, :].rearrange("(j p) f -> p j f", p=128))
                nc.sync.dma_start(t[:SH - full, TPC - 1, :], shard[full:, :])
                return t

            # ---- one GCN layer ----
            def do_layer(lidx, tabs, elem, fcn, w_rhs, b_sb, out_shard, pool_ps,
                         pre=None):
                seg_tiles = {}
                CHs = (CHA, CHB)
                csums = (csumA, csumB)

                def seg(stream, s):
                    key = (stream, s)
                    if key not in seg_tiles:
                        nch = min(SEG, CHs[stream] - s * SEG)
                        t = gp.tile([128, SEG, elem], dt.bfloat16, tag=f"g{stream}")
                        tab = tabs[stream][:, :]
                        nc.gpsimd.dma_gather(
                            t[:, :nch, :], tab,
                            idx_sbs[stream][:, s * SEG * 8:(s * SEG + nch) * 8],
                            nch * 128, nch * 128, elem)
                        seg_tiles[key] = t
                    return seg_tiles[key]

                if pre:
                    pre(seg)

                with tc.tile_pool(name=f"psl{lidx}", bufs=2, space="PSUM") as psl:
                    if lidx == 1:
                        bb_ps = psl.tile([128, H1], dt.float32, tag="bb")
                        nc.tensor.matmul(bb_ps[:], lhsT=ones_bf[:], rhs=b1_sb[:],
                                         start=True, stop=True)
                        b1bc = cp.tile([128, H1], dt.bfloat16, tag="b1bc")
                        nc.vector.tensor_copy(b1bc[:], bb_ps[:])

                    for t in range(TPC):
                        rows = min(128, SH - t * 128)
                        chunks = [(2, t)] + \
                                 [(0, g) for g in range(csumA[t], csumA[t + 1])] + \
                                 [(1, g) for g in range(csumB[t], csumB[t + 1])]
                        aggs = [psl.tile([128, 128], dt.float32, tag=f"agg{fc}", name=f"agg{fc}")
                                for fc in range(fcn)]
                        for i, (st, gc) in enumerate(chunks):
                            if st == 2:
                                lh = slabs[lidx]
                                lh_ap = lambda fc: lh[:, gc, fc * 128:(fc + 1) * 128]
                                oc = (CHA + CHB + gc) * 128
                            else:
                                sgt = seg(st, gc // SEG)
                                col = gc % SEG
                                lh_ap = lambda fc, s=sgt, co=col: s[:, co, fc * 128:(fc + 1) * 128]
                                oc = (gc if st == 0 else CHA + gc) * 128
                            for fc in range(fcn):
                                nc.tensor.matmul(
                                    aggs[fc][:], lhsT=lh_ap(fc),
                                    rhs=oh_all[:, oc:oc + 128],
                                    start=(i == 0), stop=(i == len(chunks) - 1))
                        if lidx == 1:
                            agg_f32 = wp.tile([128, 128], dt.float32, tag="aggf32")
                            nc.scalar.activation(agg_f32[:], aggs[0][:], AF.Copy)
                            # h1 tile = relu(aggT^T + b1): transpose + bias + relu
                            tr_ps = psl.tile([128, 128], dt.float32, tag="tr")
                            nc.tensor.transpose(tr_ps[:], agg_f32[:], ident[:])
                            h_sb = wp.tile([128, H1], dt.bfloat16, tag="hsb")
                            nc.vector.tensor_tensor(h_sb[:], tr_ps[:], b1bc[:], OP.add)
                            nc.vector.tensor_scalar(h_sb[:], h_sb[:], 0.0, None, OP.max)
                            nc.sync.dma_start(out_shard[t * 128:t * 128 + rows, :],
                                              h_sb[:rows])
                        else:
                            agg_sbs = []
                            for fc in range(fcn):
                                a = wp.tile([128, 128], dt.bfloat16, tag=f"aggsb{fc}",
                                            name=f"aggsb{fc}")
                                nc.scalar.activation(a[:], aggs[fc][:], AF.Copy)
                                agg_sbs.append(a)
                            fout = H2 if lidx == 2 else H3
                            h_ps = psl.tile([128, fout], dt.float32, tag="hps")
                            nc.tensor.matmul(h_ps[:], lhsT=ones_bf[:], rhs=b_sb[:],
                                             start=True, stop=False)
                            for fc in range(fcn):
                                nc.tensor.matmul(h_ps[:], lhsT=agg_sbs[fc][:],
                                                 rhs=w_rhs(fc),
                                                 start=False, stop=(fc == fcn - 1))
                            h_sb = wp.tile([128, fout], dt.bfloat16, tag="hsb")
                            nc.scalar.activation(h_sb[:], h_ps[:], AF.Relu)
                            if lidx == 2:
                                nc.sync.dma_start(
                                    out_shard[t * 128:t * 128 + rows, :], h_sb[:rows])
                            else:
                                ohb = wp.tile([128, 128], dt.bfloat16, tag="ohb")
                                nc.vector.tensor_scalar(
                                    ohb[:], iota_sb[:], bl_sb[:, t:t + 1], None,
                                    OP.is_equal)
                                nc.tensor.matmul(pool_ps[:], lhsT=ohb[:], rhs=h_sb[:],
                                                 start=(t == 0), stop=(t == TPC - 1))

            def ag(in_ap, out_t):
                nc.gpsimd.collective_compute(
                    "AllGather", mybir.AluOpType.bypass,
                    replica_groups=[list(range(NC))],
                    ins=[in_ap.opt() if hasattr(in_ap, 'opt') else in_ap],
                    outs=[out_t.opt()])

            HS2 = SH // 2
            if STAGE >= 2:
                do_layer(1, (z1A, z1B), H1, 1, None, b1_sb, h1_shard, None)
            if STAGE >= 3:
                ag(h1_shard[:HS2, :], h1A)
            if STAGE >= 4:
                slabs[2] = load_slab(2, h1_shard, H1)

                def pre2(seg):
                    seg(0, 0)
                    seg(0, 1)
                    ag(h1_shard[HS2:, :], h1B)

                do_layer(2, (h1A, h1B), H1, 1, lambda fc: w2_sb[:], b2_sb, h2_shard,
                         None, pre=pre2)
                ag(h2_shard[:HS2, :], h2A)
            if STAGE >= 5:
                pool_ps = pps.tile([128, H3], dt.float32)
                slabs[3] = load_slab(3, h2_shard, H2)

                def pre3(seg):
                    seg(0, 0)
                    seg(0, 1)
                    ag(h2_shard[HS2:, :], h2B)

                do_layer(3, (h2A, h2B), H2, 2, lambda fc: w3_sb[:, fc, :], b3_sb,
                         None, pool_ps, pre=pre3)
                pool_sb = wp.tile([128, H3], dt.float32, tag="poolsb")
                nc.vector.tensor_scalar(pool_sb[:], pool_ps[:], cinv_sb[:, :1], None, OP.mult)
                nc.sync.dma_start(pool_shard[:], pool_sb[:])
            if STAGE >= 6:
                ag(pool_shard, pool_all)

            # ---- assemble pooledT [512, 256] and run the FC head ----
            if STAGE < 6:
                dummy = wp.tile([128, 1], dt.float32, tag="dummy")
                nc.gpsimd.memset(dummy[:], 0.0)
                nc.sync.dma_start(out_ext[0:128, :], dummy[:])
                nc.sync.dma_start(out_ext[128:256, :], dummy[:])
            if STAGE >= 6:
              with tc.tile_pool(name="psf", bufs=2, space="PSUM") as psf:
                  poolT = []
                  for fc in range(4):
                      pt = cp.tile([128, G], dt.float32, tag=f"poolT{fc}")
                      nc.gpsimd.memset(pt[:], 0.0)
                      poolT.append(pt)
                  for c in range(NC):
                      pc_sb = wp.tile([128, H3], dt.float32, tag="pc", bufs=2)
                      nc.sync.dma_start(pc_sb[:], pool_all[c * 128:(c + 1) * 128, :])
                      wcols = min(128, G - g0s[c])
                      for fc in range(4):
                          tp_ps = psf.tile([128, 128], dt.float32, tag="tp", bufs=4)
                          nc.tensor.transpose(tp_ps[:], pc_sb[:, fc * 128:(fc + 1) * 128],
                                              ident[:])
                          sl = poolT[fc][:, g0s[c]:g0s[c] + wcols]
                          nc.vector.tensor_tensor(sl, sl, tp_ps[:, :wcols], OP.add)
                  for gh in range(2):
                      fc_ps = psf.tile([128, 1], dt.float32, tag="fcps")
                      nc.tensor.matmul(fc_ps[:], lhsT=ones_f32[:], rhs=bfc_sb[:],
                                       start=True, stop=False)
                      for fc in range(4):
                          nc.tensor.matmul(fc_ps[:],
                                           lhsT=poolT[fc][:, gh * 128:(gh + 1) * 128],
                                           rhs=wfc_sb[:, fc:fc + 1],
                                           start=False, stop=(fc == 3))
                      o_sb = wp.tile([128, 1], dt.float32, tag="osb")
                      nc.scalar.activation(o_sb[:], fc_ps[:], AF.Sigmoid)
                      nc.sync.dma_start(out_ext[gh * 128:(gh + 1) * 128, :], o_sb[:])

    nc.compile()
    return nc


def _install_profile_hook():
    """Provide antenv.axon_hooks (NTFF profiling) if the image lacks it."""
    import importlib
    try:
        importlib.import_module("antenv.axon_hooks")
        return
    except ImportError:
        pass
    import types
    import ctypes
    import contextlib
    so_path = "/opt/axon/libaxon_pjrt.so"
    mod = types.ModuleType("antenv.axon_hooks")
    _state = {"hook": None}

    def set_axon_ntff_profile_hook(h):
        _state["hook"] = h

    def get_axon_ntff_profile_hook():
        if _state["hook"] is None and os.path.exists(so_path):
            lib = ctypes.CDLL(so_path)
            if hasattr(lib, "axon_start_nrt_profile"):
                lib.axon_start_nrt_profile.argtypes = [
                    ctypes.POINTER(ctypes.c_int64), ctypes.c_size_t]
                lib.axon_start_nrt_profile.restype = ctypes.c_int64
                lib.axon_stop_nrt_profile.argtypes = [ctypes.c_char_p]
                lib.axon_stop_nrt_profile.restype = ctypes.c_int64

                @contextlib.contextmanager
                def _hook(output_dir, device_ids):
                    import jax
                    jax.devices()
                    if device_ids:
                        ids = (ctypes.c_int64 * len(device_ids))(*device_ids)
                        rc = lib.axon_start_nrt_profile(ids, len(device_ids))
                    else:
                        rc = lib.axon_start_nrt_profile(None, 0)
                    if rc != 0:
                        raise RuntimeError(f"axon_start_nrt_profile rc={rc}")
                    try:
                        yield
                    finally:
                        n = lib.axon_stop_nrt_profile(str(output_dir).encode())
                        print(f"profile: {n} file(s) written to {output_dir}")

                _state["hook"] = _hook
        return _state["hook"]

    mod.set_axon_ntff_profile_hook = set_axon_ntff_profile_hook
    mod.get_axon_ntff_profile_hook = get_axon_ntff_profile_hook
    sys.modules["antenv.axon_hooks"] = mod


def kernel(**inputs):
    global LAST_EXEC_NS
    from concourse.bass_utils import run_bass_kernel_spmd

    per_core, struct = _prep(inputs["x"], inputs["edge_index"],
                             inputs["edge_weight"], inputs["batch"])

    key = (struct["CHA"], struct["CHB"], tuple(struct["csumA"]),
           tuple(struct["csumB"]), tuple(struct["g0s"]))
    if key not in _CACHE:
        _CACHE[key] = _build(struct)
    nc = _CACHE[key]

    x = np.asarray(inputs["x"], np.float32)
    W1 = np.asarray(inputs["W1"], np.float32)
    W2 = np.asarray(inputs["W2"], np.float32)
    W3 = np.asarray(inputs["W3"], np.float32)
    Wfc = np.asarray(inputs["Wfc"], np.float32)
    shared = dict(
        xT=np.ascontiguousarray(x.T).astype(bf16),
        w1=W1.astype(bf16),
        w2=W2.astype(bf16),
        w3=np.ascontiguousarray(W3.reshape(2, 128, H3).transpose(1, 0, 2)).astype(bf16),
        wfc=np.ascontiguousarray(Wfc.reshape(4, 128).T).astype(np.float32),
        b1=np.asarray(inputs["b1"], np.float32).reshape(1, H1).astype(bf16),
        b2=np.asarray(inputs["b2"], np.float32).reshape(1, H2).astype(bf16),
        b3=np.asarray(inputs["b3"], np.float32).reshape(1, H3).astype(bf16),
        bfc=np.asarray(inputs["bfc"], np.float32).reshape(1, 1),
        iota=np.ascontiguousarray(
            np.tile(np.arange(128, dtype=np.float32)[None, :], (128, 1))).astype(bf16),
    )
    in_maps = [{**shared, **pc} for pc in per_core]

    if TRACE:
        _install_profile_hook()
    res = run_bass_kernel_spmd(nc, in_maps, list(range(NC)), trace=TRACE)
    LAST_EXEC_NS = res.exec_time_ns
    return res.results[0]["out"]



# revision 7
# speedup vs baseline: 1.3539x; 1.3539x over previous
"""GCN (3-layer + global mean pool + FC/sigmoid) on 8 Trainium2 NeuronCores.

Node-sharded graph partitioning: nodes split into 8 contiguous shards of
6250; each core owns its shard's incident edges. Aggregation is
aggregate-first (A @ h, then @ W): feature rows are fetched with
dma_gather (int16 indices over two 25000-row table halves), scatter-added
via one-hot matmuls on the tensor engine. Layer 1 aggregates the 8-wide
input x directly from host-prepped padded tables (no device-side x@W1
phase). Edge slots are packed globally per stream (chunks may span two
dst tiles), with group-aligned chunk boundaries so the block schedule is
uniform across the 8 SPMD cores. Halo exchange is two AllGathers per
layer (shard halves), issued early inside the tile loop; layer L+1's
A-stream is gathered upfront into a persistent SBUF table so it never
queues behind the B-half AllGather. The head reduces pooled features
against Wfc per-core, AllGathers 512B of scalars, and assembles the
global output with shift matmuls.
"""
import sys
import os

for _p in ("/opt/trn_rl_repo", "/root/.axon_site/_ro/trn_rl_repo"):
    if os.path.isdir(_p) and _p not in sys.path:
        sys.path.append(_p)

import numpy as np
import ml_dtypes

bf16 = ml_dtypes.bfloat16

N = 50000
E = 150000
G = 256
NC = 8
SH = N // NC             # 6250 nodes per core
TPC = (SH + 127) // 128  # 49 tiles per core (last tile has 106 nodes)
HS2 = SH // 2            # 3125
HALF = N // 2            # 25000-row table halves (int16-indexable)
H1, H2, H3 = 128, 256, 512
GSZ = int(os.environ.get('KGSZ', '7'))   # tiles per chunk-alignment group
SEG = int(os.environ.get('KSEG', '8'))   # chunks per dma_gather call
AG_AT = int(os.environ.get('KAGAT', '26'))  # tile index to issue half-A AG

TRACE = False
LAST_EXEC_NS = None
_CACHE = {}


def _prep(x, edge_index, edge_weight, batch):
    """Host-side graph preprocessing -> per-core metadata arrays."""
    x = np.asarray(x, np.float32)
    ei = np.asarray(edge_index)
    src = ei[0].astype(np.int64)
    dst = ei[1].astype(np.int64)
    w = np.asarray(edge_weight, np.float32)
    batch = np.asarray(batch).astype(np.int64)

    deg = np.bincount(dst, weights=w, minlength=N).astype(np.float32) + 1.0
    dinv = (1.0 / np.sqrt(deg)).astype(np.float32)
    norm = (dinv[src] * w * dinv[dst]).astype(np.float32)
    norm_self = (dinv * dinv).astype(np.float32)

    core = dst // SH
    local = dst % SH
    tile = local // 128
    srco = src // SH
    srcl = src % SH
    half = (srcl >= HS2).astype(np.int64)
    src_row = srco * HS2 + np.where(half == 1, srcl - HS2, srcl)
    NG = (TPC + GSZ - 1) // GSZ
    group = tile // GSZ

    # chunk counts per (stream, group): uniform max over cores
    key_cg = (core * 2 + half) * NG + group
    cnt_chg = np.bincount(key_cg, minlength=NC * 2 * NG).reshape(NC, 2, NG)
    CHg = ((cnt_chg + 127) // 128).max(axis=0)          # [2, NG]
    chunk_base = np.zeros((2, NG + 1), np.int64)
    chunk_base[:, 1:] = np.cumsum(CHg, axis=1)
    CH = [int(CHg[0].sum()), int(CHg[1].sum())]

    # slot assignment: edges sorted by (core, half, dst local, src row);
    # each (core, half, group) run packs back-to-back from its group's
    # chunk base.
    order = np.lexsort((src_row, local, half, core))
    sc = core[order]
    sh_ = half[order]
    sg = group[order]
    sl = local[order]
    st = tile[order]
    srow = src_row[order]
    snorm = norm[order]

    key = (sc * 2 + sh_) * NG + sg
    run_start = np.zeros(NC * 2 * NG + 1, np.int64)
    run_start[1:] = np.cumsum(np.bincount(key, minlength=NC * 2 * NG))
    offs = np.arange(len(key)) - run_start[key]
    slot = chunk_base[(sh_, sg)] * 128 + offs
    kchunk = slot // 128
    spos = slot % 128

    # per-(stream, tile) chunk spans: min/max over all cores' edges
    k_first = np.full((2, TPC), 1 << 30, np.int64)
    k_last = np.full((2, TPC), -1, np.int64)
    np.minimum.at(k_first, (sh_, st), kchunk)
    np.maximum.at(k_last, (sh_, st), kchunk)

    # oh block layout: per tile t: [A-span blocks][B-span blocks][self]
    CHmax = max(CH)
    colmap = np.full((2, CHmax, TPC), -1, np.int64)
    sched = []
    selfcol = np.zeros(TPC, np.int64)
    col = 0
    for t in range(TPC):
        ent = []
        for h in (0, 1):
            if k_last[h, t] >= 0:
                for k in range(int(k_first[h, t]), int(k_last[h, t]) + 1):
                    colmap[h, k, t] = col
                    ent.append((h, int(k), col))
                    col += 128
        selfcol[t] = col
        col += 128
        sched.append(ent)
    BLK = col // 128

    cntg = np.bincount(batch, minlength=G).astype(np.float32)
    cntinv_g = (1.0 / np.maximum(cntg, 1.0)).astype(np.float32)
    g0s = [int(batch[c * SH]) for c in range(NC)]
    for c in range(NC):
        assert int(batch[(c + 1) * SH - 1]) - g0s[c] < 128, "graph window > 128"

    # head shift blocks: nonzero (core, graph-half) pairs
    head_blocks = []
    shift_mats = []
    for c in range(NC):
        for gh in range(2):
            pd = g0s[c] + np.arange(128) - gh * 128
            val = (pd >= 0) & (pd < 128) & (g0s[c] + np.arange(128) < G)
            if not val.any():
                continue
            m = np.zeros((128, 128), np.float32)
            rows = np.arange(128)[val]
            m[rows, pd[val]] = 1.0
            head_blocks.append((c, gh, len(shift_mats) * 128))
            shift_mats.append(m)
    NSH = len(shift_mats)
    shift_all = np.concatenate(shift_mats, axis=1)  # [128, NSH*128]

    def idx_pack(lin):
        a = lin.reshape(-1, 16).T
        return np.ascontiguousarray(np.tile(a, (8, 1)))

    norm_b = snorm.astype(bf16).astype(np.float32)
    per_core = []
    for c in range(NC):
        m = sc == c
        idxs = []
        for h in (0, 1):
            mh = m & (sh_ == h)
            ia = np.zeros(CH[h] * 128, np.int16)
            ia[slot[mh]] = srow[mh].astype(np.int16)
            idxs.append(idx_pack(ia))
        oh = np.zeros((128, BLK * 128), np.float32)
        cols = colmap[(sh_[m], kchunk[m], st[m])] + (sl[m] % 128)
        assert (colmap[(sh_[m], kchunk[m], st[m])] >= 0).all()
        oh[spos[m], cols] = norm_b[m]
        # self-loop diagonal blocks
        pr = np.arange(128)
        for t in range(TPC):
            nrows = min(128, SH - t * 128)
            nodes = c * SH + t * 128 + pr[:nrows]
            oh[pr[:nrows], selfcol[t] + pr[:nrows]] = norm_self[nodes]

        xs = np.zeros((128, TPC * 8), np.float32)
        for t in range(TPC):
            nrows = min(128, SH - t * 128)
            xs[:nrows, t * 8:(t + 1) * 8] = x[c * SH + t * 128:
                                              c * SH + t * 128 + nrows]

        bl = np.full((TPC * 128,), -1.0, np.float32)
        bl[:SH] = (batch[c * SH:(c + 1) * SH] - g0s[c]).astype(np.float32)
        ig = g0s[c] + np.arange(128)
        cinv = np.where(ig < G, cntinv_g[np.minimum(ig, G - 1)], 0.0)

        per_core.append(dict(
            idxA=idxs[0], idxB=idxs[1],
            ohall=np.ascontiguousarray(oh).astype(bf16),
            xself=xs.astype(bf16),
            batchloc=np.ascontiguousarray(bl.reshape(TPC, 128).T),
            cntinv=cinv.astype(np.float32).reshape(128, 1),
        ))

    # shared padded-x gather tables (row r of half h = node
    # (r//HS2)*SH + h*HS2 + r%HS2, padded 8 -> 128 cols)
    xpads = []
    for h in (0, 1):
        xp = np.zeros((HALF, 128), np.float32)
        rows = (np.arange(HALF) // HS2) * SH + h * HS2 + np.arange(HALF) % HS2
        xp[:, :8] = x[rows]
        xpads.append(xp.astype(bf16))

    struct = dict(
        CHA=CH[0], CHB=CH[1], BLK=BLK, NSH=NSH,
        sched=tuple(tuple(e) for e in sched),
        selfcol=tuple(int(v) for v in selfcol),
        head_blocks=tuple(head_blocks),
        g0s=tuple(g0s),
    )
    shared_host = dict(xA=xpads[0], xB=xpads[1], shiftm=shift_all)
    return per_core, shared_host, struct


def _build(struct):
    import concourse.bacc as bacc
    import concourse.mybir as mybir
    import concourse.tile as tile
    from concourse.masks import make_identity

    dt = mybir.dt
    AF = mybir.ActivationFunctionType
    OP = mybir.AluOpType

    CHA, CHB = struct["CHA"], struct["CHB"]
    BLK, NSH = struct["BLK"], struct["NSH"]
    sched = struct["sched"]
    selfcol = struct["selfcol"]
    head_blocks = struct["head_blocks"]
    CHs = (CHA, CHB)

    nc = bacc.Bacc("TRN2", target_bir_lowering=False, debug=False,
                   num_devices=NC)

    xA_in = nc.dram_tensor("xA", [HALF, 128], dt.bfloat16, kind="ExternalInput")
    xB_in = nc.dram_tensor("xB", [HALF, 128], dt.bfloat16, kind="ExternalInput")
    w1_in = nc.dram_tensor("w1", [8, H1], dt.bfloat16, kind="ExternalInput")
    w2_in = nc.dram_tensor("w2", [H1, H2], dt.bfloat16, kind="ExternalInput")
    w3_in = nc.dram_tensor("w3", [128, 2, H3], dt.bfloat16, kind="ExternalInput")
    wfc_in = nc.dram_tensor("wfc", [128, 4], dt.float32, kind="ExternalInput")
    b1_in = nc.dram_tensor("b1", [1, H1], dt.bfloat16, kind="ExternalInput")
    b2_in = nc.dram_tensor("b2", [1, H2], dt.bfloat16, kind="ExternalInput")
    b3_in = nc.dram_tensor("b3", [1, H3], dt.bfloat16, kind="ExternalInput")
    bfc_in = nc.dram_tensor("bfc", [1, 1], dt.float32, kind="ExternalInput")
    iota_in = nc.dram_tensor("iota", [128, 128], dt.bfloat16, kind="ExternalInput")
    idxA_in = nc.dram_tensor("idxA", [128, CHA * 8], dt.int16, kind="ExternalInput")
    idxB_in = nc.dram_tensor("idxB", [128, CHB * 8], dt.int16, kind="ExternalInput")
    oh_in = nc.dram_tensor("ohall", [128, BLK * 128], dt.bfloat16,
                           kind="ExternalInput")
    xself_in = nc.dram_tensor("xself", [128, TPC * 8], dt.bfloat16,
                              kind="ExternalInput")
    bl_in = nc.dram_tensor("batchloc", [128, TPC], dt.float32, kind="ExternalInput")
    cinv_in = nc.dram_tensor("cntinv", [128, 1], dt.float32, kind="ExternalInput")
    shift_in = nc.dram_tensor("shiftm", [128, NSH * 128], dt.float32,
                              kind="ExternalInput")
    out_ext = nc.dram_tensor("out", [G, 1], dt.float32, kind="ExternalOutput")

    with tile.TileContext(nc) as tc:
        with tc.tile_pool(name="const", bufs=1) as cp, \
             tc.tile_pool(name="meta", bufs=1) as mp, \
             tc.tile_pool(name="work", bufs=3) as wp, \
             tc.tile_pool(name="slabs", bufs=1) as slp, \
             tc.tile_pool(name="pps", bufs=1, space="PSUM") as pps, \
             tc.tile_pool(name="dram", bufs=1, space="DRAM") as dram:

            def load(pool, t_in, shape, dtype, tag):
                t = pool.tile(shape, dtype, tag=tag)
                nc.sync.dma_start(t[:], t_in[:])
                return t

            iota_sb = load(cp, iota_in, [128, 128], dt.bfloat16, "iota")
            w1_sb = load(cp, w1_in, [8, H1], dt.bfloat16, "w1")
            w2_sb = load(cp, w2_in, [H1, H2], dt.bfloat16, "w2")
            w3_sb = load(cp, w3_in, [128, 2, H3], dt.bfloat16, "w3")
            wfc_sb = load(cp, wfc_in, [128, 4], dt.float32, "wfc")
            b1_sb = load(cp, b1_in, [1, H1], dt.bfloat16, "b1")
            b2_sb = load(cp, b2_in, [1, H2], dt.bfloat16, "b2")
            b3_sb = load(cp, b3_in, [1, H3], dt.bfloat16, "b3")
            bfc_sb = load(cp, bfc_in, [1, 1], dt.float32, "bfc")
            shift_sb = load(cp, shift_in, [128, NSH * 128], dt.float32, "shiftm")
            idx_sbs = [load(mp, idxA_in, [128, CHA * 8], dt.int16, "idxA"),
                       load(mp, idxB_in, [128, CHB * 8], dt.int16, "idxB")]
            # one-hot table: load in column slices so early tiles don't wait
            # on the full 10+ MB transfer
            oh_all = mp.tile([128, BLK * 128], dt.bfloat16, tag="ohall")
            NSL = 8
            slw = ((BLK + NSL - 1) // NSL) * 128
            for s in range(NSL):
                c0 = s * slw
                c1 = min(BLK * 128, c0 + slw)
                if c0 < c1:
                    nc.sync.dma_start(oh_all[:, c0:c1], oh_in[:, c0:c1])
            bl_sb = load(mp, bl_in, [128, TPC], dt.float32, "bl")
            cinv_sb = load(mp, cinv_in, [128, 1], dt.float32, "cinv")

            ones_bf = cp.tile([1, 128], dt.bfloat16, tag="ones_bf")
            nc.vector.memset(ones_bf[:], 1.0)
            ones_f32 = cp.tile([1, 128], dt.float32, tag="ones_f32")
            nc.vector.memset(ones_f32[:], 1.0)
            ident = cp.tile([128, 128], dt.float32, tag="ident")
            make_identity(nc, ident[:])

            slab1 = slp.tile([128, TPC, 8], dt.bfloat16, tag="slab1")
            nc.sync.dma_start(slab1[:], xself_in[:].rearrange(
                "p (t f) -> p t f", f=8))
            slab2 = slp.tile([128, TPC, H1], dt.bfloat16, tag="slab2")
            nc.vector.memset(slab2[:, TPC - 1, :], 0.0)
            slab3 = slp.tile([128, TPC, H2], dt.bfloat16, tag="slab3")
            nc.vector.memset(slab3[:, TPC - 1, :], 0.0)
            slabs = {1: slab1, 2: slab2, 3: slab3}

            h1_shard = dram.tile([SH, H1], dt.bfloat16, tag="h1s")
            h1A = dram.tile([HALF, H1], dt.bfloat16, tag="h1A")
            h1B = dram.tile([HALF, H1], dt.bfloat16, tag="h1B")
            h2_shard = dram.tile([SH, H2], dt.bfloat16, tag="h2s")
            h2A = dram.tile([HALF, H2], dt.bfloat16, tag="h2A")
            h2B = dram.tile([HALF, H2], dt.bfloat16, tag="h2B")
            s_shard = dram.tile([128, 1], dt.float32, tag="ss")
            s_all = dram.tile([NC * 128, 1], dt.float32, tag="sa")

            def ag(in_ap, out_t):
                nc.gpsimd.collective_compute(
                    "AllGather", mybir.AluOpType.bypass,
                    replica_groups=[list(range(NC))],
                    ins=[in_ap.opt() if hasattr(in_ap, 'opt') else in_ap],
                    outs=[out_t.opt()])

            pool_ps = pps.tile([128, H3], dt.float32)

            def do_layer(lidx, tabs, elem, out_shard, agA, agB, gp):
                fcn = elem // 128
                seg_tiles = {}
                gA = None
                if lidx > 1:
                    # A-stream gathered upfront into a persistent table so
                    # its gathers never queue behind the half-B AllGather
                    gA = gp.tile([128, CHA, elem], dt.bfloat16,
                                 tag="gA", bufs=1)
                    for s in range((CHA + SEG - 1) // SEG):
                        nch = min(SEG, CHA - s * SEG)
                        nc.gpsimd.dma_gather(
                            gA[:, s * SEG:s * SEG + nch, :], tabs[0][:, :],
                            idx_sbs[0][:, s * SEG * 8:(s * SEG + nch) * 8],
                            nch * 128, nch * 128, elem)

                def seg(stream, s):
                    key = (stream, s)
                    if key not in seg_tiles:
                        nch = min(SEG, CHs[stream] - s * SEG)
                        t = gp.tile([128, SEG, elem], dt.bfloat16,
                                    tag=f"g{stream}")
                        nc.gpsimd.dma_gather(
                            t[:, :nch, :], tabs[stream][:, :],
                            idx_sbs[stream][:, s * SEG * 8:(s * SEG + nch) * 8],
                            nch * 128, nch * 128, elem)
                        seg_tiles[key] = t
                    return seg_tiles[key]

                def lh_ap(st, k, fc):
                    if st == 2:
                        if lidx == 1:
                            return slabs[1][:, k, :]
                        return slabs[lidx][:, k, fc * 128:(fc + 1) * 128]
                    if st == 0 and gA is not None:
                        return gA[:, k, fc * 128:(fc + 1) * 128]
                    sgt = seg(st, k // SEG)
                    co = k % SEG
                    if lidx == 1:
                        return sgt[:, co, :8]
                    return sgt[:, co, fc * 128:(fc + 1) * 128]

                with tc.tile_pool(name=f"psl{lidx}", bufs=2, space="PSUM") as psl:
                    for t in range(TPC):
                        rows = min(128, SH - t * 128)
                        chain = [(2, t, selfcol[t])] + list(sched[t])
                        if lidx == 1:
                            agg = psl.tile([8, 128], dt.float32, tag="agg0")
                            for i, (stm, k, col) in enumerate(chain):
                                nc.tensor.matmul(
                                    agg[:], lhsT=lh_ap(stm, k, 0),
                                    rhs=oh_all[:, col:col + 128],
                                    start=(i == 0), stop=(i == len(chain) - 1))
                            aggx_sb = wp.tile([8, 128], dt.bfloat16, tag="aggx")
                            if t % 2 == 0:
                                nc.vector.tensor_copy(aggx_sb[:], agg[:])
                            else:
                                nc.scalar.activation(aggx_sb[:], agg[:], AF.Copy)
                            h_ps = psl.tile([128, H1], dt.float32, tag="hps")
                            nc.tensor.matmul(h_ps[:], lhsT=ones_bf[:],
                                             rhs=b1_sb[:], start=True, stop=False)
                            nc.tensor.matmul(h_ps[:], lhsT=aggx_sb[:],
                                             rhs=w1_sb[:], start=False, stop=True)
                            nc.scalar.activation(slab2[:rows, t, :], h_ps[:rows],
                                                 AF.Relu)
                            nc.sync.dma_start(
                                out_shard[t * 128:t * 128 + rows, :],
                                slab2[:rows, t, :])
                        else:
                            fout = H2 if lidx == 2 else H3
                            aggs = [psl.tile([128, 128], dt.float32,
                                             tag=f"agg{fc}", name=f"agg{fc}")
                                    for fc in range(fcn)]
                            for i, (stm, k, col) in enumerate(chain):
                                for fc in range(fcn):
                                    nc.tensor.matmul(
                                        aggs[fc][:], lhsT=lh_ap(stm, k, fc),
                                        rhs=oh_all[:, col:col + 128],
                                        start=(i == 0),
                                        stop=(i == len(chain) - 1))
                            agg_sbs = []
                            for fc in range(fcn):
                                a = wp.tile([128, 128], dt.bfloat16,
                                            tag=f"aggsb{fc}", name=f"aggsb{fc}")
                                if (t + fc) % 2 == 0:
                                    nc.vector.tensor_copy(a[:], aggs[fc][:])
                                else:
                                    nc.scalar.activation(a[:], aggs[fc][:],
                                                         AF.Copy)
                                agg_sbs.append(a)
                            h_ps = psl.tile([128, fout], dt.float32, tag="hps")
                            nc.tensor.matmul(h_ps[:], lhsT=ones_bf[:],
                                             rhs=(b2_sb[:] if lidx == 2
                                                  else b3_sb[:]),
                                             start=True, stop=False)
                            for fc in range(fcn):
                                rhs = (w2_sb[:] if lidx == 2
                                       else w3_sb[:, fc, :])
                                nc.tensor.matmul(h_ps[:], lhsT=agg_sbs[fc][:],
                                                 rhs=rhs, start=False,
                                                 stop=(fc == fcn - 1))
                            if lidx == 2:
                                nc.scalar.activation(slab3[:rows, t, :],
                                                     h_ps[:rows], AF.Relu)
                                nc.sync.dma_start(
                                    out_shard[t * 128:t * 128 + rows, :],
                                    slab3[:rows, t, :])
                            else:
                                h_sb = wp.tile([128, H3], dt.bfloat16,
                                               tag="hsb")
                                nc.scalar.activation(h_sb[:], h_ps[:], AF.Relu)
                                ohb = wp.tile([128, 128], dt.bfloat16,
                                              tag="ohb")
                                nc.vector.tensor_scalar(
                                    ohb[:], iota_sb[:], bl_sb[:, t:t + 1],
                                    None, OP.is_equal)
                                nc.tensor.matmul(pool_ps[:], lhsT=ohb[:],
                                                 rhs=h_sb[:], start=(t == 0),
                                                 stop=(t == TPC - 1))
                        if agA is not None and t == AG_AT:
                            ag(out_shard[:HS2, :], agA)
                        if agA is not None and t == TPC - 1:
                            ag(out_shard[HS2:, :], agB)

            with tc.tile_pool(name="gl1", bufs=2) as gp1:
                do_layer(1, (xA_in, xB_in), 128, h1_shard, h1A, h1B, gp1)
            with tc.tile_pool(name="gl2", bufs=2) as gp2:
                do_layer(2, (h1A, h1B), 128, h2_shard, h2A, h2B, gp2)
            with tc.tile_pool(name="gl3", bufs=2) as gp3:
                do_layer(3, (h2A, h2B), 256, None, None, None, gp3)

            # ---- head: pooled/cnt -> @Wfc per-core -> 512B AG -> assemble ----
            with tc.tile_pool(name="psf", bufs=2, space="PSUM") as psf:
                pool_sb = wp.tile([128, H3], dt.float32, tag="poolsb")
                nc.vector.tensor_scalar(pool_sb[:], pool_ps[:],
                                        cinv_sb[:, :1], None, OP.mult)
                s_ps = psf.tile([128, 1], dt.float32, tag="sps")
                for fc in range(4):
                    tr_ps = psf.tile([128, 128], dt.float32, tag="tr", bufs=2)
                    nc.tensor.transpose(tr_ps[:],
                                        pool_sb[:, fc * 128:(fc + 1) * 128],
                                        ident[:])
                    ptf = wp.tile([128, 128], dt.float32, tag="ptf", bufs=2)
                    nc.vector.tensor_copy(ptf[:], tr_ps[:])
                    nc.tensor.matmul(s_ps[:], lhsT=ptf[:],
                                     rhs=wfc_sb[:, fc:fc + 1],
                                     start=(fc == 0), stop=(fc == 3))
                s_sb = wp.tile([128, 1], dt.float32, tag="ssb")
                nc.scalar.activation(s_sb[:], s_ps[:], AF.Copy)
                nc.sync.dma_start(s_shard[:], s_sb[:])
                ag(s_shard, s_all)

                sall_sb = wp.tile([128, NC], dt.float32, tag="sall")
                nc.sync.dma_start(sall_sb[:],
                                  s_all[:].rearrange("(c p) o -> p (c o)", p=128))
                for gh in range(2):
                    blks = [hb for hb in head_blocks if hb[1] == gh]
                    o_ps = psf.tile([128, 1], dt.float32, tag="ops", bufs=2)
                    nc.tensor.matmul(o_ps[:], lhsT=ones_f32[:], rhs=bfc_sb[:],
                                     start=True, stop=(len(blks) == 0))
                    for i, (c, _, scol) in enumerate(blks):
                        nc.tensor.matmul(o_ps[:],
                                         lhsT=shift_sb[:, scol:scol + 128],
                                         rhs=sall_sb[:, c:c + 1],
                                         start=False, stop=(i == len(blks) - 1))
                    o_sb = wp.tile([128, 1], dt.float32, tag="osb", bufs=2)
                    nc.scalar.activation(o_sb[:], o_ps[:], AF.Sigmoid)
                    nc.sync.dma_start(out_ext[gh * 128:(gh + 1) * 128, :],
                                      o_sb[:])

    nc.compile()
    return nc


def _install_profile_hook():
    """Provide antenv.axon_hooks (NTFF profiling) if the image lacks it."""
    import importlib
    try:
        importlib.import_module("antenv.axon_hooks")
        return
    except ImportError:
        pass
    import types
    import ctypes
    import contextlib
    so_path = "/opt/axon/libaxon_pjrt.so"
    mod = types.ModuleType("antenv.axon_hooks")
    _state = {"hook": None}

    def set_axon_ntff_profile_hook(h):
        _state["hook"] = h

    def get_axon_ntff_profile_hook():
        if _state["hook"] is None and os.path.exists(so_path):
            lib = ctypes.CDLL(so_path)
            if hasattr(lib, "axon_start_nrt_profile"):
                lib.axon_start_nrt_profile.argtypes = [
                    ctypes.POINTER(ctypes.c_int64), ctypes.c_size_t]
                lib.axon_start_nrt_profile.restype = ctypes.c_int64
                lib.axon_stop_nrt_profile.argtypes = [ctypes.c_char_p]
                lib.axon_stop_nrt_profile.restype = ctypes.c_int64

                @contextlib.contextmanager
                def _hook(output_dir, device_ids):
                    import jax
                    jax.devices()
                    if device_ids:
                        ids = (ctypes.c_int64 * len(device_ids))(*device_ids)
                        rc = lib.axon_start_nrt_profile(ids, len(device_ids))
                    else:
                        rc = lib.axon_start_nrt_profile(None, 0)
                    if rc != 0:
                        raise RuntimeError(f"axon_start_nrt_profile rc={rc}")
                    try:
                        yield
                    finally:
                        n = lib.axon_stop_nrt_profile(str(output_dir).encode())
                        print(f"profile: {n} file(s) written to {output_dir}")

                _state["hook"] = _hook
        return _state["hook"]

    mod.set_axon_ntff_profile_hook = set_axon_ntff_profile_hook
    mod.get_axon_ntff_profile_hook = get_axon_ntff_profile_hook
    sys.modules["antenv.axon_hooks"] = mod


def kernel(**inputs):
    global LAST_EXEC_NS
    from concourse.bass_utils import run_bass_kernel_spmd

    per_core, shared_host, struct = _prep(
        inputs["x"], inputs["edge_index"], inputs["edge_weight"],
        inputs["batch"])

    key = (struct["CHA"], struct["CHB"], struct["BLK"], struct["sched"],
           struct["selfcol"], struct["head_blocks"], struct["g0s"])
    if key not in _CACHE:
        _CACHE[key] = _build(struct)
    nc = _CACHE[key]

    W1 = np.asarray(inputs["W1"], np.float32)
    W2 = np.asarray(inputs["W2"], np.float32)
    W3 = np.asarray(inputs["W3"], np.float32)
    Wfc = np.asarray(inputs["Wfc"], np.float32)
    shared = dict(
        xA=shared_host["xA"],
        xB=shared_host["xB"],
        shiftm=shared_host["shiftm"],
        w1=W1.astype(bf16),
        w2=W2.astype(bf16),
        w3=np.ascontiguousarray(
            W3.reshape(2, 128, H3).transpose(1, 0, 2)).astype(bf16),
        wfc=np.ascontiguousarray(Wfc.reshape(4, 128).T).astype(np.float32),
        b1=np.asarray(inputs["b1"], np.float32).reshape(1, H1).astype(bf16),
        b2=np.asarray(inputs["b2"], np.float32).reshape(1, H2).astype(bf16),
        b3=np.asarray(inputs["b3"], np.float32).reshape(1, H3).astype(bf16),
        bfc=np.asarray(inputs["bfc"], np.float32).reshape(1, 1),
        iota=np.ascontiguousarray(
            np.tile(np.arange(128, dtype=np.float32)[None, :],
                    (128, 1))).astype(bf16),
    )
    in_maps = [{**shared, **pc} for pc in per_core]

    if TRACE:
        _install_profile_hook()
    res = run_bass_kernel_spmd(nc, in_maps, list(range(NC)), trace=TRACE)
    LAST_EXEC_NS = res.exec_time_ns
    return res.results[0]["out"]


# revision 18
# speedup vs baseline: 1.3811x; 1.0201x over previous
"""GCN (3-layer + global mean pool + FC/sigmoid) on 8 Trainium2 NeuronCores.

Node-sharded graph partitioning: nodes split into 8 contiguous shards of
6250; each core owns its shard's incident edges. Aggregation is
aggregate-first (A @ h, then @ W): feature rows are fetched with
dma_gather (int16 indices over two 25000-row table halves), scatter-added
via one-hot matmuls on the tensor engine. Layer 1 aggregates the 8-wide
input x directly from host-prepped padded tables (no device-side x@W1
phase). Edge slots are packed globally per stream (chunks may span two
dst tiles), with group-aligned chunk boundaries so the block schedule is
uniform across the 8 SPMD cores. Halo exchange is two AllGathers per
layer (shard halves), issued early inside the tile loop; layer L+1's
A-stream is gathered upfront into a persistent SBUF table so it never
queues behind the B-half AllGather. The head reduces pooled features
against Wfc per-core, AllGathers 512B of scalars, and assembles the
global output with shift matmuls.
"""
import sys
import os

for _p in ("/opt/trn_rl_repo", "/root/.axon_site/_ro/trn_rl_repo"):
    if os.path.isdir(_p) and _p not in sys.path:
        sys.path.append(_p)

import numpy as np
import ml_dtypes

bf16 = ml_dtypes.bfloat16

N = 50000
E = 150000
G = 256
NC = 8
SH = N // NC             # 6250 nodes per core
TPC = (SH + 127) // 128  # 49 tiles per core (last tile has 106 nodes)
HS2 = SH // 2            # 3125
HALF = N // 2            # 25000-row table halves (int16-indexable)
H1, H2, H3 = 128, 256, 512
GSZ = int(os.environ.get('KGSZ', '7'))   # tiles per chunk-alignment group
SEG = int(os.environ.get('KSEG', '8'))   # chunks per dma_gather call
AG_AT = int(os.environ.get('KAGAT', '26'))  # tile index to issue half-A AG

TRACE = False
LAST_EXEC_NS = None
_CACHE = {}


def _prep(x, edge_index, edge_weight, batch):
    """Host-side graph preprocessing -> per-core metadata arrays."""
    x = np.asarray(x, np.float32)
    ei = np.asarray(edge_index)
    src = ei[0].astype(np.int64)
    dst = ei[1].astype(np.int64)
    w = np.asarray(edge_weight, np.float32)
    batch = np.asarray(batch).astype(np.int64)

    deg = np.bincount(dst, weights=w, minlength=N).astype(np.float32) + 1.0
    dinv = (1.0 / np.sqrt(deg)).astype(np.float32)
    norm = (dinv[src] * w * dinv[dst]).astype(np.float32)
    norm_self = (dinv * dinv).astype(np.float32)

    core = dst // SH
    local = dst % SH
    tile = local // 128
    srco = src // SH
    srcl = src % SH
    half = (srcl >= HS2).astype(np.int64)
    src_row = srco * HS2 + np.where(half == 1, srcl - HS2, srcl)
    NG = (TPC + GSZ - 1) // GSZ
    group = tile // GSZ

    # chunk counts per (stream, group): uniform max over cores
    key_cg = (core * 2 + half) * NG + group
    cnt_chg = np.bincount(key_cg, minlength=NC * 2 * NG).reshape(NC, 2, NG)
    CHg = ((cnt_chg + 127) // 128).max(axis=0)          # [2, NG]
    chunk_base = np.zeros((2, NG + 1), np.int64)
    chunk_base[:, 1:] = np.cumsum(CHg, axis=1)
    CH = [int(CHg[0].sum()), int(CHg[1].sum())]

    # slot assignment: edges sorted by (core, half, tile, src row) — src
    # ascending within a tile segment keeps gather source addresses
    # monotonic for the DGE; each (core, half, group) run packs
    # back-to-back from its group's chunk base.
    order = np.lexsort((src_row, tile, half, core))
    sc = core[order]
    sh_ = half[order]
    sg = group[order]
    sl = local[order]
    st = tile[order]
    srow = src_row[order]
    snorm = norm[order]

    key = (sc * 2 + sh_) * NG + sg
    run_start = np.zeros(NC * 2 * NG + 1, np.int64)
    run_start[1:] = np.cumsum(np.bincount(key, minlength=NC * 2 * NG))
    offs = np.arange(len(key)) - run_start[key]
    slot = chunk_base[(sh_, sg)] * 128 + offs
    kchunk = slot // 128
    spos = slot % 128

    # per-(stream, tile) chunk spans: min/max over all cores' edges
    k_first = np.full((2, TPC), 1 << 30, np.int64)
    k_last = np.full((2, TPC), -1, np.int64)
    np.minimum.at(k_first, (sh_, st), kchunk)
    np.maximum.at(k_last, (sh_, st), kchunk)

    # oh block layout: per tile t: [A-span blocks][B-span blocks][self]
    CHmax = max(CH)
    colmap = np.full((2, CHmax, TPC), -1, np.int64)
    sched = []
    selfcol = np.zeros(TPC, np.int64)
    col = 0
    for t in range(TPC):
        ent = []
        for h in (0, 1):
            if k_last[h, t] >= 0:
                for k in range(int(k_first[h, t]), int(k_last[h, t]) + 1):
                    colmap[h, k, t] = col
                    ent.append((h, int(k), col))
                    col += 128
        selfcol[t] = col
        col += 128
        sched.append(ent)
    BLK = col // 128

    cntg = np.bincount(batch, minlength=G).astype(np.float32)
    cntinv_g = (1.0 / np.maximum(cntg, 1.0)).astype(np.float32)
    g0s = [int(batch[c * SH]) for c in range(NC)]
    for c in range(NC):
        assert int(batch[(c + 1) * SH - 1]) - g0s[c] < 128, "graph window > 128"

    # head shift blocks: nonzero (core, graph-half) pairs
    head_blocks = []
    shift_mats = []
    for c in range(NC):
        for gh in range(2):
            pd = g0s[c] + np.arange(128) - gh * 128
            val = (pd >= 0) & (pd < 128) & (g0s[c] + np.arange(128) < G)
            if not val.any():
                continue
            m = np.zeros((128, 128), np.float32)
            rows = np.arange(128)[val]
            m[rows, pd[val]] = 1.0
            head_blocks.append((c, gh, len(shift_mats) * 128))
            shift_mats.append(m)
    NSH = len(shift_mats)
    shift_all = np.concatenate(shift_mats, axis=1)  # [128, NSH*128]

    def idx_pack(lin):
        a = lin.reshape(-1, 16).T
        return np.ascontiguousarray(np.tile(a, (8, 1)))

    norm_b = snorm.astype(bf16).astype(np.float32)
    per_core = []
    for c in range(NC):
        m = sc == c
        idxs = []
        for h in (0, 1):
            mh = m & (sh_ == h)
            ia = np.zeros(CH[h] * 128, np.int16)
            ia[slot[mh]] = srow[mh].astype(np.int16)
            idxs.append(idx_pack(ia))
        oh = np.zeros((128, BLK * 128), np.float32)
        cols = colmap[(sh_[m], kchunk[m], st[m])] + (sl[m] % 128)
        assert (colmap[(sh_[m], kchunk[m], st[m])] >= 0).all()
        oh[spos[m], cols] = norm_b[m]
        # self-loop diagonal blocks
        pr = np.arange(128)
        for t in range(TPC):
            nrows = min(128, SH - t * 128)
            nodes = c * SH + t * 128 + pr[:nrows]
            oh[pr[:nrows], selfcol[t] + pr[:nrows]] = norm_self[nodes]

        xs = np.zeros((128, TPC * 8), np.float32)
        for t in range(TPC):
            nrows = min(128, SH - t * 128)
            xs[:nrows, t * 8:(t + 1) * 8] = x[c * SH + t * 128:
                                              c * SH + t * 128 + nrows]

        bl = np.full((TPC * 128,), -1.0, np.float32)
        bl[:SH] = (batch[c * SH:(c + 1) * SH] - g0s[c]).astype(np.float32)
        ig = g0s[c] + np.arange(128)
        cinv = np.where(ig < G, cntinv_g[np.minimum(ig, G - 1)], 0.0)

        per_core.append(dict(
            idxA=idxs[0], idxB=idxs[1],
            ohall=np.ascontiguousarray(oh).astype(bf16),
            xself=xs.astype(bf16),
            batchloc=np.ascontiguousarray(bl.reshape(TPC, 128).T),
            cntinv=cinv.astype(np.float32).reshape(128, 1),
        ))

    # shared padded-x gather tables (row r of half h = node
    # (r//HS2)*SH + h*HS2 + r%HS2, padded 8 -> 128 cols)
    xpads = []
    for h in (0, 1):
        xp = np.zeros((HALF, 128), np.float32)
        rows = (np.arange(HALF) // HS2) * SH + h * HS2 + np.arange(HALF) % HS2
        xp[:, :8] = x[rows]
        xpads.append(xp.astype(bf16))

    struct = dict(
        CHA=CH[0], CHB=CH[1], BLK=BLK, NSH=NSH,
        sched=tuple(tuple(e) for e in sched),
        selfcol=tuple(int(v) for v in selfcol),
        head_blocks=tuple(head_blocks),
        g0s=tuple(g0s),
    )
    shared_host = dict(xA=xpads[0], xB=xpads[1], shiftm=shift_all)
    return per_core, shared_host, struct


def _build(struct):
    import concourse.bacc as bacc
    import concourse.mybir as mybir
    import concourse.tile as tile
    from concourse.masks import make_identity

    dt = mybir.dt
    AF = mybir.ActivationFunctionType
    OP = mybir.AluOpType

    CHA, CHB = struct["CHA"], struct["CHB"]
    BLK, NSH = struct["BLK"], struct["NSH"]
    sched = struct["sched"]
    selfcol = struct["selfcol"]
    head_blocks = struct["head_blocks"]
    CHs = (CHA, CHB)

    nc = bacc.Bacc("TRN2", target_bir_lowering=False, debug=False,
                   num_devices=NC)

    xA_in = nc.dram_tensor("xA", [HALF, 128], dt.bfloat16, kind="ExternalInput")
    xB_in = nc.dram_tensor("xB", [HALF, 128], dt.bfloat16, kind="ExternalInput")
    w1_in = nc.dram_tensor("w1", [8, H1], dt.bfloat16, kind="ExternalInput")
    w2_in = nc.dram_tensor("w2", [H1, H2], dt.bfloat16, kind="ExternalInput")
    w3_in = nc.dram_tensor("w3", [128, 2, H3], dt.bfloat16, kind="ExternalInput")
    wfc_in = nc.dram_tensor("wfc", [128, 4], dt.float32, kind="ExternalInput")
    bbc_in = nc.dram_tensor("bbc", [128, H1 + H2 + H3], dt.bfloat16,
                            kind="ExternalInput")
    bfc_in = nc.dram_tensor("bfc", [1, 1], dt.float32, kind="ExternalInput")
    iota_in = nc.dram_tensor("iota", [128, 128], dt.bfloat16, kind="ExternalInput")
    idxA_in = nc.dram_tensor("idxA", [128, CHA * 8], dt.int16, kind="ExternalInput")
    idxB_in = nc.dram_tensor("idxB", [128, CHB * 8], dt.int16, kind="ExternalInput")
    oh_in = nc.dram_tensor("ohall", [128, BLK * 128], dt.bfloat16,
                           kind="ExternalInput")
    xself_in = nc.dram_tensor("xself", [128, TPC * 8], dt.bfloat16,
                              kind="ExternalInput")
    bl_in = nc.dram_tensor("batchloc", [128, TPC], dt.float32, kind="ExternalInput")
    cinv_in = nc.dram_tensor("cntinv", [128, 1], dt.float32, kind="ExternalInput")
    shift_in = nc.dram_tensor("shiftm", [128, NSH * 128], dt.float32,
                              kind="ExternalInput")
    out_ext = nc.dram_tensor("out", [G, 1], dt.float32, kind="ExternalOutput")

    with tile.TileContext(nc) as tc:
        with tc.tile_pool(name="const", bufs=1) as cp, \
             tc.tile_pool(name="meta", bufs=1) as mp, \
             tc.tile_pool(name="work", bufs=3) as wp, \
             tc.tile_pool(name="slabs", bufs=1) as slp, \
             tc.tile_pool(name="pps", bufs=1, space="PSUM") as pps, \
             tc.tile_pool(name="dram", bufs=1, space="DRAM") as dram:

            def load(pool, t_in, shape, dtype, tag):
                t = pool.tile(shape, dtype, tag=tag)
                nc.sync.dma_start(t[:], t_in[:])
                return t

            iota_sb = load(cp, iota_in, [128, 128], dt.bfloat16, "iota")
            w1_sb = load(cp, w1_in, [8, H1], dt.bfloat16, "w1")
            w2_sb = load(cp, w2_in, [H1, H2], dt.bfloat16, "w2")
            w3_sb = load(cp, w3_in, [128, 2, H3], dt.bfloat16, "w3")
            wfc_sb = load(cp, wfc_in, [128, 4], dt.float32, "wfc")
            bbc_sb = load(cp, bbc_in, [128, H1 + H2 + H3], dt.bfloat16, "bbc")
            bfc_sb = load(cp, bfc_in, [1, 1], dt.float32, "bfc")
            shift_sb = load(cp, shift_in, [128, NSH * 128], dt.float32, "shiftm")
            idx_sbs = [load(mp, idxA_in, [128, CHA * 8], dt.int16, "idxA"),
                       load(mp, idxB_in, [128, CHB * 8], dt.int16, "idxB")]
            # one-hot table: load in column slices so early tiles don't wait
            # on the full 10+ MB transfer
            oh_all = mp.tile([128, BLK * 128], dt.bfloat16, tag="ohall")
            NSL = 8
            slw = ((BLK + NSL - 1) // NSL) * 128
            for s in range(NSL):
                c0 = s * slw
                c1 = min(BLK * 128, c0 + slw)
                if c0 < c1:
                    nc.sync.dma_start(oh_all[:, c0:c1], oh_in[:, c0:c1])
            bl_sb = load(mp, bl_in, [128, TPC], dt.float32, "bl")
            cinv_sb = load(mp, cinv_in, [128, 1], dt.float32, "cinv")

            ones_f32 = cp.tile([1, 128], dt.float32, tag="ones_f32")
            nc.vector.memset(ones_f32[:], 1.0)
            ident = cp.tile([128, 128], dt.float32, tag="ident")
            make_identity(nc, ident[:])

            slab1 = slp.tile([128, TPC, 8], dt.bfloat16, tag="slab1")
            nc.sync.dma_start(slab1[:], xself_in[:].rearrange(
                "p (t f) -> p t f", f=8))
            slab2 = slp.tile([128, TPC, H1], dt.bfloat16, tag="slab2")
            nc.vector.memset(slab2[:, TPC - 1, :], 0.0)
            slab3 = slp.tile([128, TPC, H2], dt.bfloat16, tag="slab3")
            nc.vector.memset(slab3[:, TPC - 1, :], 0.0)
            slabs = {1: slab1, 2: slab2, 3: slab3}

            h1_shard = dram.tile([SH, H1], dt.bfloat16, tag="h1s")
            h1A = dram.tile([HALF, H1], dt.bfloat16, tag="h1A")
            h1B = dram.tile([HALF, H1], dt.bfloat16, tag="h1B")
            h2_shard = dram.tile([SH, H2], dt.bfloat16, tag="h2s")
            h2A = dram.tile([HALF, H2], dt.bfloat16, tag="h2A")
            h2B = dram.tile([HALF, H2], dt.bfloat16, tag="h2B")
            s_shard = dram.tile([128, 1], dt.float32, tag="ss")
            s_all = dram.tile([NC * 128, 1], dt.float32, tag="sa")

            def ag(in_ap, out_t):
                nc.gpsimd.collective_compute(
                    "AllGather", mybir.AluOpType.bypass,
                    replica_groups=[list(range(NC))],
                    ins=[in_ap.opt() if hasattr(in_ap, 'opt') else in_ap],
                    outs=[out_t.opt()])

            pool_ps = pps.tile([128, H3], dt.float32)

            def mk_seg(lidx, tabs, elem, gp, seg_tiles):
                def seg(stream, s):
                    key = (stream, s)
                    if key not in seg_tiles:
                        nch = min(SEG, CHs[stream] - s * SEG)
                        t = gp.tile([128, SEG, elem], dt.bfloat16,
                                    tag=f"g{stream}")
                        nc.gpsimd.dma_gather(
                            t[:, :nch, :], tabs[stream][:, :],
                            idx_sbs[stream][:, s * SEG * 8:(s * SEG + nch) * 8],
                            nch * 128, nch * 128, elem)
                        seg_tiles[key] = t
                    return seg_tiles[key]
                return seg

            def do_layer1(out_shard, agA, agB, gp):
                seg = mk_seg(1, (xA_in, xB_in), 128, gp, {})
                with tc.tile_pool(name="psl1", bufs=2, space="PSUM") as psl:
                    for t in range(TPC):
                        rows = min(128, SH - t * 128)
                        chain = [(2, t, selfcol[t])] + list(sched[t])
                        agg = psl.tile([8, 128], dt.float32, tag="agg0")
                        for i, (stm, k, col) in enumerate(chain):
                            lh = (slab1[:, k, :] if stm == 2
                                  else seg(stm, k // SEG)[:, k % SEG, :8])
                            nc.tensor.matmul(
                                agg[:], lhsT=lh,
                                rhs=oh_all[:, col:col + 128],
                                start=(i == 0), stop=(i == len(chain) - 1))
                        aggx_sb = wp.tile([8, 128], dt.bfloat16, tag="aggx")
                        if t % 2 == 0:
                            nc.vector.tensor_copy(aggx_sb[:], agg[:])
                        else:
                            nc.scalar.activation(aggx_sb[:], agg[:], AF.Copy)
                        h_ps = psl.tile([128, H1], dt.float32, tag="hps")
                        nc.tensor.matmul(h_ps[:], lhsT=aggx_sb[:],
                                         rhs=w1_sb[:], start=True, stop=True)
                        nc.vector.tensor_tensor(slab2[:rows, t, :], h_ps[:rows],
                                                bbc_sb[:rows, :H1], OP.add)
                        nc.vector.tensor_scalar(slab2[:rows, t, :],
                                                slab2[:rows, t, :], 0.0, None,
                                                OP.max)
                        nc.sync.dma_start(
                            out_shard[t * 128:t * 128 + rows, :],
                            slab2[:rows, t, :])
                        if t == AG_AT:
                            ag(out_shard[:HS2, :], agA)
                        if t == TPC - 1:
                            ag(out_shard[HS2:, :], agB)

            def do_layer23(lidx, tabs, elem, out_shard, agA, agB, gp):
                fcn = elem // 128
                fout = H2 if lidx == 2 else H3
                w_rhs = (lambda fc: w2_sb[:]) if lidx == 2 else \
                        (lambda fc: w3_sb[:, fc, :])
                bc0, bc1 = (H1, H1 + H2) if lidx == 2 else \
                           (H1 + H2, H1 + H2 + H3)
                seg = mk_seg(lidx, tabs, elem, gp, {})
                slab = slabs[lidx]
                aggA_sbs = {}
                # loop 1: self + A-stream accumulation for every tile (PE
                # stays busy while the B-half AllGather is in flight)
                with tc.tile_pool(name=f"psa{lidx}", bufs=2, space="PSUM") as psa:
                    for t in range(TPC):
                        chainA = [(2, t, selfcol[t])] + \
                                 [e for e in sched[t] if e[0] == 0]
                        aggs = [psa.tile([128, 128], dt.float32,
                                         tag=f"aggA{fc}", name=f"aggA{fc}")
                                for fc in range(fcn)]
                        for i, (stm, k, col) in enumerate(chainA):
                            for fc in range(fcn):
                                lh = (slab[:, k, fc * 128:(fc + 1) * 128]
                                      if stm == 2 else
                                      seg(stm, k // SEG)[:, k % SEG,
                                                         fc * 128:(fc + 1) * 128])
                                nc.tensor.matmul(
                                    aggs[fc][:], lhsT=lh,
                                    rhs=oh_all[:, col:col + 128],
                                    start=(i == 0), stop=(i == len(chainA) - 1))
                        for fc in range(fcn):
                            a = gp.tile([128, 128], dt.bfloat16,
                                        tag=f"asb{t}_{fc}", bufs=1)
                            if (t + fc) % 2 == 0:
                                nc.vector.tensor_copy(a[:], aggs[fc][:])
                            else:
                                nc.scalar.activation(a[:], aggs[fc][:], AF.Copy)
                            aggA_sbs[(t, fc)] = a
                # loop 2: B-stream + weight matmul + output
                with tc.tile_pool(name=f"psb{lidx}", bufs=2, space="PSUM") as psb:
                    for t in range(TPC):
                        rows = min(128, SH - t * 128)
                        chainB = [e for e in sched[t] if e[0] == 1]
                        asums = []
                        if chainB:
                            aggs = [psb.tile([128, 128], dt.float32,
                                             tag=f"aggB{fc}", name=f"aggB{fc}")
                                    for fc in range(fcn)]
                            for i, (stm, k, col) in enumerate(chainB):
                                for fc in range(fcn):
                                    lh = seg(stm, k // SEG)[:, k % SEG,
                                                            fc * 128:(fc + 1) * 128]
                                    nc.tensor.matmul(
                                        aggs[fc][:], lhsT=lh,
                                        rhs=oh_all[:, col:col + 128],
                                        start=(i == 0),
                                        stop=(i == len(chainB) - 1))
                            for fc in range(fcn):
                                a = wp.tile([128, 128], dt.bfloat16,
                                            tag=f"bsum{fc}", name=f"bsum{fc}")
                                nc.vector.tensor_tensor(
                                    a[:], aggs[fc][:],
                                    aggA_sbs[(t, fc)][:], OP.add)
                                asums.append(a)
                        else:
                            asums = [aggA_sbs[(t, fc)] for fc in range(fcn)]
                        h_ps = psb.tile([128, fout], dt.float32, tag="hps")
                        for fc in range(fcn):
                            nc.tensor.matmul(h_ps[:], lhsT=asums[fc][:],
                                             rhs=w_rhs(fc), start=(fc == 0),
                                             stop=(fc == fcn - 1))
                        if lidx == 2:
                            nc.vector.tensor_tensor(slab3[:rows, t, :],
                                                    h_ps[:rows],
                                                    bbc_sb[:rows, bc0:bc1],
                                                    OP.add)
                            nc.vector.tensor_scalar(slab3[:rows, t, :],
                                                    slab3[:rows, t, :], 0.0,
                                                    None, OP.max)
                            nc.sync.dma_start(
                                out_shard[t * 128:t * 128 + rows, :],
                                slab3[:rows, t, :])
                        else:
                            h_sb = wp.tile([128, H3], dt.bfloat16, tag="hsb")
                            nc.vector.tensor_tensor(h_sb[:], h_ps[:],
                                                    bbc_sb[:, bc0:bc1],
                                                    OP.add)
                            nc.vector.tensor_scalar(h_sb[:], h_sb[:], 0.0,
                                                    None, OP.max)
                            ohb = wp.tile([128, 128], dt.bfloat16, tag="ohb")
                            nc.vector.tensor_scalar(
                                ohb[:], iota_sb[:], bl_sb[:, t:t + 1],
                                None, OP.is_equal)
                            nc.tensor.matmul(pool_ps[:], lhsT=ohb[:],
                                             rhs=h_sb[:], start=(t == 0),
                                             stop=(t == TPC - 1))
                        if agA is not None and t == AG_AT:
                            ag(out_shard[:HS2, :], agA)
                        if agA is not None and t == TPC - 1:
                            ag(out_shard[HS2:, :], agB)

            with tc.tile_pool(name="gl1", bufs=2) as gp1:
                do_layer1(h1_shard, h1A, h1B, gp1)
            with tc.tile_pool(name="gl2", bufs=2) as gp2:
                do_layer23(2, (h1A, h1B), 128, h2_shard, h2A, h2B, gp2)
            with tc.tile_pool(name="gl3", bufs=2) as gp3:
                do_layer23(3, (h2A, h2B), 256, None, None, None, gp3)

            # ---- head: pooled/cnt -> @Wfc per-core -> 512B AG -> assemble ----
            with tc.tile_pool(name="psf", bufs=2, space="PSUM") as psf:
                pool_sb = wp.tile([128, H3], dt.float32, tag="poolsb")
                nc.vector.tensor_scalar(pool_sb[:], pool_ps[:],
                                        cinv_sb[:, :1], None, OP.mult)
                s_ps = psf.tile([128, 1], dt.float32, tag="sps")
                for fc in range(4):
                    tr_ps = psf.tile([128, 128], dt.float32, tag="tr", bufs=2)
                    nc.tensor.transpose(tr_ps[:],
                                        pool_sb[:, fc * 128:(fc + 1) * 128],
                                        ident[:])
                    ptf = wp.tile([128, 128], dt.float32, tag="ptf", bufs=2)
                    nc.vector.tensor_copy(ptf[:], tr_ps[:])
                    nc.tensor.matmul(s_ps[:], lhsT=ptf[:],
                                     rhs=wfc_sb[:, fc:fc + 1],
                                     start=(fc == 0), stop=(fc == 3))
                s_sb = wp.tile([128, 1], dt.float32, tag="ssb")
                nc.scalar.activation(s_sb[:], s_ps[:], AF.Copy)
                nc.sync.dma_start(s_shard[:], s_sb[:])
                ag(s_shard, s_all)

                sall_sb = wp.tile([128, NC], dt.float32, tag="sall")
                nc.sync.dma_start(sall_sb[:],
                                  s_all[:].rearrange("(c p) o -> p (c o)", p=128))
                for gh in range(2):
                    blks = [hb for hb in head_blocks if hb[1] == gh]
                    o_ps = psf.tile([128, 1], dt.float32, tag="ops", bufs=2)
                    nc.tensor.matmul(o_ps[:], lhsT=ones_f32[:], rhs=bfc_sb[:],
                                     start=True, stop=(len(blks) == 0))
                    for i, (c, _, scol) in enumerate(blks):
                        nc.tensor.matmul(o_ps[:],
                                         lhsT=shift_sb[:, scol:scol + 128],
                                         rhs=sall_sb[:, c:c + 1],
                                         start=False, stop=(i == len(blks) - 1))
                    o_sb = wp.tile([128, 1], dt.float32, tag="osb", bufs=2)
                    nc.scalar.activation(o_sb[:], o_ps[:], AF.Sigmoid)
                    nc.sync.dma_start(out_ext[gh * 128:(gh + 1) * 128, :],
                                      o_sb[:])

    nc.compile()
    return nc


def _install_profile_hook():
    """Provide antenv.axon_hooks (NTFF profiling) if the image lacks it."""
    import importlib
    try:
        importlib.import_module("antenv.axon_hooks")
        return
    except ImportError:
        pass
    import types
    import ctypes
    import contextlib
    so_path = "/opt/axon/libaxon_pjrt.so"
    mod = types.ModuleType("antenv.axon_hooks")
    _state = {"hook": None}

    def set_axon_ntff_profile_hook(h):
        _state["hook"] = h

    def get_axon_ntff_profile_hook():
        if _state["hook"] is None and os.path.exists(so_path):
            lib = ctypes.CDLL(so_path)
            if hasattr(lib, "axon_start_nrt_profile"):
                lib.axon_start_nrt_profile.argtypes = [
                    ctypes.POINTER(ctypes.c_int64), ctypes.c_size_t]
                lib.axon_start_nrt_profile.restype = ctypes.c_int64
                lib.axon_stop_nrt_profile.argtypes = [ctypes.c_char_p]
                lib.axon_stop_nrt_profile.restype = ctypes.c_int64

                @contextlib.contextmanager
                def _hook(output_dir, device_ids):
                    import jax
                    jax.devices()
                    if device_ids:
                        ids = (ctypes.c_int64 * len(device_ids))(*device_ids)
                        rc = lib.axon_start_nrt_profile(ids, len(device_ids))
                    else:
                        rc = lib.axon_start_nrt_profile(None, 0)
                    if rc != 0:
                        raise RuntimeError(f"axon_start_nrt_profile rc={rc}")
                    try:
                        yield
                    finally:
                        n = lib.axon_stop_nrt_profile(str(output_dir).encode())
                        print(f"profile: {n} file(s) written to {output_dir}")

                _state["hook"] = _hook
        return _state["hook"]

    mod.set_axon_ntff_profile_hook = set_axon_ntff_profile_hook
    mod.get_axon_ntff_profile_hook = get_axon_ntff_profile_hook
    sys.modules["antenv.axon_hooks"] = mod


def kernel(**inputs):
    global LAST_EXEC_NS
    from concourse.bass_utils import run_bass_kernel_spmd

    per_core, shared_host, struct = _prep(
        inputs["x"], inputs["edge_index"], inputs["edge_weight"],
        inputs["batch"])

    key = (struct["CHA"], struct["CHB"], struct["BLK"], struct["sched"],
           struct["selfcol"], struct["head_blocks"], struct["g0s"])
    if key not in _CACHE:
        _CACHE[key] = _build(struct)
    nc = _CACHE[key]

    W1 = np.asarray(inputs["W1"], np.float32)
    W2 = np.asarray(inputs["W2"], np.float32)
    W3 = np.asarray(inputs["W3"], np.float32)
    Wfc = np.asarray(inputs["Wfc"], np.float32)
    shared = dict(
        xA=shared_host["xA"],
        xB=shared_host["xB"],
        shiftm=shared_host["shiftm"],
        w1=W1.astype(bf16),
        w2=W2.astype(bf16),
        w3=np.ascontiguousarray(
            W3.reshape(2, 128, H3).transpose(1, 0, 2)).astype(bf16),
        wfc=np.ascontiguousarray(Wfc.reshape(4, 128).T).astype(np.float32),
        bbc=np.ascontiguousarray(np.tile(np.concatenate([
            np.asarray(inputs["b1"], np.float32).reshape(-1),
            np.asarray(inputs["b2"], np.float32).reshape(-1),
            np.asarray(inputs["b3"], np.float32).reshape(-1),
        ])[None, :], (128, 1))).astype(bf16),
        bfc=np.asarray(inputs["bfc"], np.float32).reshape(1, 1),
        iota=np.ascontiguousarray(
            np.tile(np.arange(128, dtype=np.float32)[None, :],
                    (128, 1))).astype(bf16),
    )
    in_maps = [{**shared, **pc} for pc in per_core]

    if TRACE:
        _install_profile_hook()
    res = run_bass_kernel_spmd(nc, in_maps, list(range(NC)), trace=TRACE)
    LAST_EXEC_NS = res.exec_time_ns
    return res.results[0]["out"]
